# revision 6
# baseline (speedup 1.0000x reference)
"""Trainium2 Bass kernel for windowed/sparse attention (nn_Attention_21732534518476).

Strategy:
  - 8 NeuronCores, one attention head per core (HEADS == 8).
  - Host-side input prep ("sharding"): transpose x -> xT, slice per-head
    projection weights, gather+transpose the relative-position bias table into
    a per-head dense [j, i] bf16 matrix, augment w_out with b_out/8.
  - On-device per core: qkv projection producing Q (q replicated at partition
    bases 0 and 32, vT at base 64) and K (k replicated at bases 0 and 32);
    transposed-score attention (scores kept [j, i] so the softmax reduction is
    a matmul with an appended ones-column on v); bias injected into PSUM via
    identity matmul; exp on the Scalar engine; unnormalized attn@v; late
    normalization; per-head output projection producing a partial outT [c, i].
    Score matmuls for the two i-halves run row-tiled (array rows 0-31 / 32-63)
    and the two attn@v matmuls run col-tiled (cols 0-32 / 64-96) so the K=32
    and M=33 matmuls overlap in the PE array.
  - Host sums the 8 partial outputs (the head reduction) and reshapes.

All matmul operands are bf16 (fp32 PSUM accumulation); exp input is fp32.
"""

import os
import sys

sys.path.insert(0, "/opt/trn_rl_repo")
os.environ.setdefault("MYCRO_LOCAL_CACHE", "1")

import numpy as np
import ml_dtypes

BF = ml_dtypes.bfloat16

B, N, C = 4, 2048, 256
HEADS, D = 8, 32
BN = B * N  # 8192
JT = 16  # j chunks of 128 per batch
IB = 16  # i blocks of 512 over the full 8192
SCALE = D ** -0.5

_CACHE = {}


def _build():
    from concourse import bass, mybir, bacc
    import concourse.tile as tile
    from concourse.masks import make_identity

    f32 = mybir.dt.float32
    bfl = mybir.dt.bfloat16
    Exp = mybir.ActivationFunctionType.Exp
    mult = mybir.AluOpType.mult

    nc = bacc.Bacc(
        "TRN2",
        target_bir_lowering=False,
        debug=False,
        num_devices=8,
    )

    xt_ext = nc.dram_tensor("xt", [128, 2, BN], bfl, kind="ExternalInput")
    # projection weights, lhsT layout [c(128), cc, m]: qv cols [q,q,vT], k cols [k,k]
    wqv_ext = nc.dram_tensor("wqv", [128, 2, 96], bfl, kind="ExternalInput")
    wk_ext = nc.dram_tensor("wk", [128, 2, 64], bfl, kind="ExternalInput")
    biast_ext = nc.dram_tensor("biast", [128, JT, N], bfl, kind="ExternalInput")
    wout_ext = nc.dram_tensor("wout", [33, 256], bfl, kind="ExternalInput")
    out_ext = nc.dram_tensor("out", [128, 2, BN], f32, kind="ExternalOutput")

    with tile.TileContext(nc) as tc:
        with (
            tc.tile_pool(name="const", bufs=1) as constp,
            tc.tile_pool(name="big", bufs=1) as bigp,
            tc.tile_pool(name="ptp", bufs=3) as ptp,
            tc.tile_pool(name="outp", bufs=2) as outp,
            tc.tile_pool(name="small", bufs=3) as smallp,
            tc.tile_pool(name="pst", bufs=2, space="PSUM") as pst,
            tc.tile_pool(name="po", bufs=4, space="PSUM") as po,
        ):
            ident = constp.tile([128, 128], bfl, tag="ident")
            make_identity(nc, ident[:])
            wqv_sb = constp.tile([128, 2, 96], bfl, tag="wqv")
            nc.sync.dma_start(out=wqv_sb[:], in_=wqv_ext[:])
            wk_sb = constp.tile([128, 2, 64], bfl, tag="wk")
            nc.sync.dma_start(out=wk_sb[:], in_=wk_ext[:])
            wout_sb = constp.tile([33, 256], bfl, tag="wout")
            nc.sync.dma_start(out=wout_sb[:], in_=wout_ext[:])

            xt_sb = bigp.tile([128, 2, BN], bfl, tag="xt")
            for cc in range(2):
                for q4 in range(4):
                    nc.sync.dma_start(
                        out=xt_sb[:, cc, q4 * 2048 : (q4 + 1) * 2048],
                        in_=xt_ext[:, cc, q4 * 2048 : (q4 + 1) * 2048],
                    )
            biast_sb = bigp.tile([128, JT, N], bfl, tag="biast")
            for jc in range(JT):
                nc.sync.dma_start(out=biast_sb[:, jc, :], in_=biast_ext[:, jc, :])

            # projections
            q_sb = bigp.tile([96, IB, 512], bfl, tag="q")  # rows: q@0, q@32, vT@64
            k_sb = bigp.tile([64, IB, 512], bfl, tag="k")  # rows: k@0, k@32
            for ib in range(IB):
                psq = pst.tile([96, 512], f32, tag="st")
                psk = pst.tile([64, 512], f32, tag="st")
                for cc in range(2):
                    nc.tensor.matmul(
                        psq[:],
                        lhsT=wqv_sb[:, cc, :],
                        rhs=xt_sb[:, cc, ib * 512 : (ib + 1) * 512],
                        start=(cc == 0),
                        stop=(cc == 1),
                    )
                for cc in range(2):
                    nc.tensor.matmul(
                        psk[:],
                        lhsT=wk_sb[:, cc, :],
                        rhs=xt_sb[:, cc, ib * 512 : (ib + 1) * 512],
                        start=(cc == 0),
                        stop=(cc == 1),
                    )
                nc.vector.tensor_copy(q_sb[:, ib, :], psq[:])
                nc.vector.tensor_copy(k_sb[:, ib, :], psk[:])

            # v in [j, d] layout with an appended ones column -> [128, b, jhi, 33]
            v1_sb = bigp.tile([128, B, JT, 33], bfl, tag="v1")
            nc.gpsimd.memset(v1_sb[:, :, :, 32:33], 1.0)
            for b in range(B):
                tp = po.tile([128, JT, 32], bfl, tag="o")
                for jh in range(JT):
                    j0 = b * N + jh * 128  # global j
                    ib = j0 // 512
                    off = j0 % 512
                    nc.tensor.transpose(
                        tp[:, jh, :],
                        q_sb[64:96, ib, off : off + 128],
                        ident[64:96, 64:96],
                    )
                nc.vector.tensor_copy(v1_sb[:, b, :, 0:32], tp[:])

            # attention units: (b, ic2) with i-halves A/B of 512 each
            for b in range(B):
                out_t = outp.tile([128, 2, 2048], f32, tag="out_t")
                for ic2 in range(2):
                    iA = b * 4 + ic2 * 2  # i-block index (512-wide) of half A
                    iB = iA + 1
                    o_A = po.tile([128, 512], f32, tag="o")
                    o_B = po.tile([128, 512], f32, tag="o")
                    for jc in range(JT):
                        jb = (b * N + jc * 128) // 512
                        joff = (jc * 128) % 512
                        st = pst.tile([128, 1024], f32, tag="st")
                        # bias inject (identity matmul) then scores accumulate
                        nc.tensor.matmul(
                            st[:, 0:512],
                            lhsT=ident[:],
                            rhs=biast_sb[:, jc, (iA % 4) * 512 : (iA % 4) * 512 + 512],
                            start=True,
                            stop=False,
                        )
                        nc.tensor.matmul(
                            st[:, 512:1024],
                            lhsT=ident[:],
                            rhs=biast_sb[:, jc, (iB % 4) * 512 : (iB % 4) * 512 + 512],
                            start=True,
                            stop=False,
                        )
                        # row-tiled scores: pair (0,0) and (32,0)
                        nc.tensor.matmul(
                            st[:, 0:512],
                            lhsT=k_sb[0:32, jb, joff : joff + 128],
                            rhs=q_sb[0:32, iA, :],
                            start=False,
                            stop=True,
                        )
                        nc.tensor.matmul(
                            st[:, 512:1024],
                            lhsT=k_sb[32:64, jb, joff : joff + 128],
                            rhs=q_sb[32:64, iB, :],
                            start=False,
                            stop=True,
                        )
                        pt = ptp.tile([128, 1024], bfl, tag="pt")
                        nc.scalar.activation(pt[:], st[:], Exp)
                        # col-tiled attn@v: (0,0) and (0,64)
                        nc.tensor.matmul(
                            o_A[0:33, :],
                            lhsT=v1_sb[:, b, jc, :],
                            rhs=pt[:, 0:512],
                            start=(jc == 0),
                            stop=(jc == JT - 1),
                        )
                        nc.tensor.matmul(
                            o_B[64:97, :],
                            lhsT=v1_sb[:, b, jc, :],
                            rhs=pt[:, 512:1024],
                            start=(jc == 0),
                            stop=(jc == JT - 1),
                        )
                    # tail: recip of sums, broadcast, normalize, out projection
                    recip = smallp.tile([1, 1024], f32, tag="recip")
                    nc.vector.reciprocal(recip[:, 0:512], o_A[32:33, :])
                    nc.vector.reciprocal(recip[:, 512:1024], o_B[96:97, :])
                    recipB = smallp.tile([33, 1024], f32, tag="recipB")
                    nc.gpsimd.partition_broadcast(recipB[:], recip[:])
                    onorm = smallp.tile([33, 1024], bfl, tag="onorm")
                    nc.vector.tensor_tensor(
                        onorm[:, 0:512], o_A[0:33, :], recipB[:, 0:512], mult
                    )
                    nc.vector.tensor_tensor(
                        onorm[:, 512:1024], o_B[64:97, :], recipB[:, 512:1024], mult
                    )
                    for cc in range(2):
                        for half in range(2):
                            op_ps = po.tile([128, 512], f32, tag="o")
                            nc.tensor.matmul(
                                op_ps[:],
                                lhsT=wout_sb[:, cc * 128 : (cc + 1) * 128],
                                rhs=onorm[:, half * 512 : (half + 1) * 512],
                                start=True,
                                stop=True,
                            )
                            nc.vector.tensor_copy(
                                out_t[
                                    :,
                                    cc,
                                    ic2 * 1024 + half * 512 : ic2 * 1024 + half * 512 + 512,
                                ],
                                op_ps[:],
                            )
                for cc in range(2):
                    nc.sync.dma_start(
                        out=out_ext[:, cc, b * N : (b + 1) * N],
                        in_=out_t[:, cc, :],
                    )
    nc.compile()
    return nc


def _prep_inputs(x, w_qkv, bias_table, w_out, b_out, rel_index):
    x = np.asarray(x, dtype=np.float32)
    w_qkv = np.asarray(w_qkv, dtype=np.float32)
    bias_table = np.asarray(bias_table, dtype=np.float32)
    w_out = np.asarray(w_out, dtype=np.float32)
    b_out = np.asarray(b_out, dtype=np.float32)
    rel_index = np.asarray(rel_index)

    xt = np.ascontiguousarray(
        x.reshape(BN, C).T.reshape(2, 128, BN).transpose(1, 0, 2)
    ).astype(BF)

    # rel transposed so the gather lands directly in [j, i] order
    relT = np.ascontiguousarray(rel_index.reshape(N, N).T).reshape(-1)

    in_maps = []
    for h in range(HEADS):
        wq = w_qkv[:, h * D : (h + 1) * D] * SCALE
        wk = w_qkv[:, C + h * D : C + (h + 1) * D]
        wv = w_qkv[:, 2 * C + h * D : 2 * C + (h + 1) * D]
        qv = np.concatenate([wq, wq, wv], axis=1)  # (256, 96)
        kk = np.concatenate([wk, wk], axis=1)  # (256, 64)
        wqv_h = np.ascontiguousarray(
            qv.reshape(2, 128, 96).transpose(1, 0, 2)
        ).astype(BF)
        wk_h = np.ascontiguousarray(
            kk.reshape(2, 128, 64).transpose(1, 0, 2)
        ).astype(BF)

        biast = bias_table[:, h][relT].reshape(N, N)  # [j, i]
        biast_h = np.ascontiguousarray(
            biast.reshape(JT, 128, N).transpose(1, 0, 2)
        ).astype(BF)

        wout_h = np.concatenate(
            [w_out[h * D : (h + 1) * D, :], (b_out / HEADS)[None, :]], axis=0
        ).astype(BF)  # (33, 256)

        in_maps.append(
            {
                "xt": xt,
                "wqv": wqv_h,
                "wk": wk_h,
                "biast": biast_h,
                "wout": np.ascontiguousarray(wout_h),
            }
        )
    return in_maps


def _run(in_maps, trace=False, **kwargs):
    from concourse.bass_utils import run_bass_kernel_spmd

    if "nc" not in _CACHE:
        _CACHE["nc"] = _build()
    nc = _CACHE["nc"]
    res = run_bass_kernel_spmd(
        nc, in_maps, core_ids=list(range(8)), trace=trace, **kwargs
    )
    return res


def kernel(x, w_qkv, bias_table, w_out, b_out, rel_index):
    in_maps = _prep_inputs(x, w_qkv, bias_table, w_out, b_out, rel_index)
    res = _run(in_maps, trace=False)
    acc = np.zeros((256, BN), dtype=np.float32)
    for c in range(8):
        o = res.results[c]["out"]  # (128, 2, 8192) f32
        acc += np.asarray(o).transpose(1, 0, 2).reshape(256, BN)
    out = acc.T.reshape(B, N, C).astype(np.float32)
    return out


# revision 9
# speedup vs baseline: 1.0931x; 1.0931x over previous
"""Trainium2 Bass kernel for windowed/sparse attention (nn_Attention_21732534518476).

Strategy:
  - 8 NeuronCores, one attention head per core (HEADS == 8).
  - Host-side input prep ("sharding"): transpose x -> xT, slice per-head
    projection weights, gather+transpose the relative-position bias table into
    a per-head dense [j, i] bf16 matrix, augment w_out with b_out/8.
  - On-device per core: qkv projection producing Q (q replicated at partition
    bases 0 and 32, vT at base 64) and K (k replicated at bases 0 and 32);
    transposed-score attention (scores kept [j, i] so the softmax reduction is
    a matmul with an appended ones-column on v); bias injected into PSUM via
    identity matmul; exp on the Scalar engine; unnormalized attn@v; late
    normalization; per-head output projection producing a partial outT [c, i].
    Score matmuls for the two i-halves run row-tiled (array rows 0-31 / 32-63)
    and the two attn@v matmuls run col-tiled (cols 0-32 / 64-96) so the K=32
    and M=33 matmuls overlap in the PE array.
  - Host sums the 8 partial outputs (the head reduction) and reshapes.

All matmul operands are bf16 (fp32 PSUM accumulation); exp input is fp32.
"""

import os
import sys

sys.path.insert(0, "/opt/trn_rl_repo")
os.environ.setdefault("MYCRO_LOCAL_CACHE", "1")

import numpy as np
import ml_dtypes

BF = ml_dtypes.bfloat16

B, N, C = 4, 2048, 256
HEADS, D = 8, 32
BN = B * N  # 8192
JT = 16  # j chunks of 128 per batch
IB = 16  # i blocks of 512 over the full 8192
SCALE = D ** -0.5

_CACHE = {}


def _build():
    from concourse import bass, mybir, bacc
    import concourse.tile as tile
    from concourse.masks import make_identity

    f32 = mybir.dt.float32
    bfl = mybir.dt.bfloat16
    Exp = mybir.ActivationFunctionType.Exp
    mult = mybir.AluOpType.mult

    nc = bacc.Bacc(
        "TRN2",
        target_bir_lowering=False,
        debug=False,
        num_devices=8,
    )

    xt_ext = nc.dram_tensor("xt", [128, 2, BN], bfl, kind="ExternalInput")
    # projection weights, lhsT layout [c(128), cc, m]: qv cols [q,q,vT], k cols [k,k]
    wqv_ext = nc.dram_tensor("wqv", [128, 2, 96], bfl, kind="ExternalInput")
    wk_ext = nc.dram_tensor("wk", [128, 2, 64], bfl, kind="ExternalInput")
    biast_ext = nc.dram_tensor("biast", [128, JT, N], bfl, kind="ExternalInput")
    wout_ext = nc.dram_tensor("wout", [33, 256], bfl, kind="ExternalInput")
    out_ext = nc.dram_tensor("out", [128, 2, BN], f32, kind="ExternalOutput")

    with tile.TileContext(nc) as tc:
        with (
            tc.tile_pool(name="const", bufs=1) as constp,
            tc.tile_pool(name="big", bufs=1) as bigp,
            tc.tile_pool(name="ptp", bufs=3) as ptp,
            tc.tile_pool(name="outp", bufs=2) as outp,
            tc.tile_pool(name="small", bufs=3) as smallp,
            tc.tile_pool(name="pst", bufs=3, space="PSUM") as pst,
            tc.tile_pool(name="po", bufs=2, space="PSUM") as po,
        ):
            ident = constp.tile([128, 128], bfl, tag="ident")
            make_identity(nc, ident[:])
            wqv_sb = constp.tile([128, 2, 96], bfl, tag="wqv")
            nc.sync.dma_start(out=wqv_sb[:], in_=wqv_ext[:])
            wk_sb = constp.tile([128, 2, 64], bfl, tag="wk")
            nc.sync.dma_start(out=wk_sb[:], in_=wk_ext[:])
            wout_sb = constp.tile([33, 256], bfl, tag="wout")
            nc.sync.dma_start(out=wout_sb[:], in_=wout_ext[:])

            xt_sb = bigp.tile([128, 2, BN], bfl, tag="xt")
            for cc in range(2):
                for q4 in range(4):
                    nc.sync.dma_start(
                        out=xt_sb[:, cc, q4 * 2048 : (q4 + 1) * 2048],
                        in_=xt_ext[:, cc, q4 * 2048 : (q4 + 1) * 2048],
                    )
            biast_sb = bigp.tile([128, JT, N], bfl, tag="biast")
            for jc in range(JT):
                nc.sync.dma_start(out=biast_sb[:, jc, :], in_=biast_ext[:, jc, :])

            # projections
            q_sb = bigp.tile([96, IB, 512], bfl, tag="q")  # rows: q@0, q@32, vT@64
            k_sb = bigp.tile([64, IB, 512], bfl, tag="k")  # rows: k@0, k@32
            for ib in range(IB):
                psq = pst.tile([96, 512], f32, tag="st")
                psk = pst.tile([64, 512], f32, tag="st")
                for cc in range(2):
                    nc.tensor.matmul(
                        psq[:],
                        lhsT=wqv_sb[:, cc, :],
                        rhs=xt_sb[:, cc, ib * 512 : (ib + 1) * 512],
                        start=(cc == 0),
                        stop=(cc == 1),
                    )
                for cc in range(2):
                    nc.tensor.matmul(
                        psk[:],
                        lhsT=wk_sb[:, cc, :],
                        rhs=xt_sb[:, cc, ib * 512 : (ib + 1) * 512],
                        start=(cc == 0),
                        stop=(cc == 1),
                    )
                nc.vector.tensor_copy(q_sb[:, ib, :], psq[:])
                nc.vector.tensor_copy(k_sb[:, ib, :], psk[:])

            # v in [j, d] layout with an appended ones column -> [128, b, jhi, 33]
            v1_sb = bigp.tile([128, B, JT, 33], bfl, tag="v1")
            nc.gpsimd.memset(v1_sb[:, :, :, 32:33], 1.0)
            for b in range(B):
                tp = po.tile([128, JT, 32], bfl, tag="o")
                for jh in range(JT):
                    j0 = b * N + jh * 128  # global j
                    ib = j0 // 512
                    off = j0 % 512
                    nc.tensor.transpose(
                        tp[:, jh, :],
                        q_sb[64:96, ib, off : off + 128],
                        ident[64:96, 64:96],
                    )
                nc.vector.tensor_copy(v1_sb[:, b, :, 0:32], tp[:])

            # attention units: (b, ic2) with i-halves A/B of 512 each
            for b in range(B):
                out_t = outp.tile([128, 2, 2048], f32, tag="out_t")
                for ic2 in range(2):
                    iA = b * 4 + ic2 * 2  # i-block index (512-wide) of half A
                    iB = iA + 1
                    o_pair = po.tile([128, 512], f32, tag="o")
                    o_A = o_pair
                    o_B = o_pair
                    for jc in range(JT):
                        jb = (b * N + jc * 128) // 512
                        joff = (jc * 128) % 512
                        st = pst.tile([128, 1024], f32, tag="st")
                        # bias inject (identity matmul) then scores accumulate
                        nc.tensor.matmul(
                            st[:, 0:512],
                            lhsT=ident[:],
                            rhs=biast_sb[:, jc, (iA % 4) * 512 : (iA % 4) * 512 + 512],
                            start=True,
                            stop=False,
                        )
                        nc.tensor.matmul(
                            st[:, 512:1024],
                            lhsT=ident[:],
                            rhs=biast_sb[:, jc, (iB % 4) * 512 : (iB % 4) * 512 + 512],
                            start=True,
                            stop=False,
                        )
                        # row-tiled scores: pair (0,0) and (32,0)
                        nc.tensor.matmul(
                            st[:, 0:512],
                            lhsT=k_sb[0:32, jb, joff : joff + 128],
                            rhs=q_sb[0:32, iA, :],
                            start=False,
                            stop=True,
                        )
                        nc.tensor.matmul(
                            st[:, 512:1024],
                            lhsT=k_sb[32:64, jb, joff : joff + 128],
                            rhs=q_sb[32:64, iB, :],
                            start=False,
                            stop=True,
                        )
                        pt = ptp.tile([128, 1024], bfl, tag="pt")
                        nc.scalar.activation(pt[:], st[:], Exp)
                        # col-tiled attn@v: (0,0) and (0,64)
                        nc.tensor.matmul(
                            o_A[0:33, :],
                            lhsT=v1_sb[:, b, jc, :],
                            rhs=pt[:, 0:512],
                            start=(jc == 0),
                            stop=(jc == JT - 1),
                        )
                        nc.tensor.matmul(
                            o_B[64:97, :],
                            lhsT=v1_sb[:, b, jc, :],
                            rhs=pt[:, 512:1024],
                            start=(jc == 0),
                            stop=(jc == JT - 1),
                            skip_group_check=True,
                        )
                    # tail: recip of sums, broadcast, normalize, out projection
                    recip = smallp.tile([1, 1024], f32, tag="recip")
                    nc.vector.reciprocal(recip[:, 0:512], o_A[32:33, :])
                    nc.vector.reciprocal(recip[:, 512:1024], o_B[96:97, :])
                    recipB = smallp.tile([33, 1024], f32, tag="recipB")
                    nc.gpsimd.partition_broadcast(recipB[:], recip[:])
                    onorm = smallp.tile([33, 1024], bfl, tag="onorm")
                    nc.vector.tensor_tensor(
                        onorm[:, 0:512], o_A[0:33, :], recipB[:, 0:512], mult
                    )
                    nc.vector.tensor_tensor(
                        onorm[:, 512:1024], o_B[64:97, :], recipB[:, 512:1024], mult
                    )
                    for cc in range(2):
                        for half in range(2):
                            op_ps = pst.tile([128, 512], f32, tag="st")
                            nc.tensor.matmul(
                                op_ps[:],
                                lhsT=wout_sb[:, cc * 128 : (cc + 1) * 128],
                                rhs=onorm[:, half * 512 : (half + 1) * 512],
                                start=True,
                                stop=True,
                            )
                            nc.vector.tensor_copy(
                                out_t[
                                    :,
                                    cc,
                                    ic2 * 1024 + half * 512 : ic2 * 1024 + half * 512 + 512,
                                ],
                                op_ps[:],
                            )
                for cc in range(2):
                    nc.sync.dma_start(
                        out=out_ext[:, cc, b * N : (b + 1) * N],
                        in_=out_t[:, cc, :],
                    )
    nc.compile()
    return nc


def _prep_inputs(x, w_qkv, bias_table, w_out, b_out, rel_index):
    x = np.asarray(x, dtype=np.float32)
    w_qkv = np.asarray(w_qkv, dtype=np.float32)
    bias_table = np.asarray(bias_table, dtype=np.float32)
    w_out = np.asarray(w_out, dtype=np.float32)
    b_out = np.asarray(b_out, dtype=np.float32)
    rel_index = np.asarray(rel_index)

    xt = np.ascontiguousarray(
        x.reshape(BN, C).T.reshape(2, 128, BN).transpose(1, 0, 2)
    ).astype(BF)

    # rel transposed so the gather lands directly in [j, i] order
    relT = np.ascontiguousarray(rel_index.reshape(N, N).T).reshape(-1)

    in_maps = []
    for h in range(HEADS):
        wq = w_qkv[:, h * D : (h + 1) * D] * SCALE
        wk = w_qkv[:, C + h * D : C + (h + 1) * D]
        wv = w_qkv[:, 2 * C + h * D : 2 * C + (h + 1) * D]
        qv = np.concatenate([wq, wq, wv], axis=1)  # (256, 96)
        kk = np.concatenate([wk, wk], axis=1)  # (256, 64)
        wqv_h = np.ascontiguousarray(
            qv.reshape(2, 128, 96).transpose(1, 0, 2)
        ).astype(BF)
        wk_h = np.ascontiguousarray(
            kk.reshape(2, 128, 64).transpose(1, 0, 2)
        ).astype(BF)

        biast = bias_table[:, h][relT].reshape(N, N)  # [j, i]
        biast_h = np.ascontiguousarray(
            biast.reshape(JT, 128, N).transpose(1, 0, 2)
        ).astype(BF)

        wout_h = np.concatenate(
            [w_out[h * D : (h + 1) * D, :], (b_out / HEADS)[None, :]], axis=0
        ).astype(BF)  # (33, 256)

        in_maps.append(
            {
                "xt": xt,
                "wqv": wqv_h,
                "wk": wk_h,
                "biast": biast_h,
                "wout": np.ascontiguousarray(wout_h),
            }
        )
    return in_maps


def _run(in_maps, trace=False, **kwargs):
    from concourse.bass_utils import run_bass_kernel_spmd

    if "nc" not in _CACHE:
        _CACHE["nc"] = _build()
    nc = _CACHE["nc"]
    res = run_bass_kernel_spmd(
        nc, in_maps, core_ids=list(range(8)), trace=trace, **kwargs
    )
    return res


def kernel(x, w_qkv, bias_table, w_out, b_out, rel_index):
    in_maps = _prep_inputs(x, w_qkv, bias_table, w_out, b_out, rel_index)
    res = _run(in_maps, trace=False)
    acc = np.zeros((256, BN), dtype=np.float32)
    for c in range(8):
        o = res.results[c]["out"]  # (128, 2, 8192) f32
        acc += np.asarray(o).transpose(1, 0, 2).reshape(256, BN)
    out = acc.T.reshape(B, N, C).astype(np.float32)
    return out


# revision 10
# speedup vs baseline: 1.1777x; 1.0774x over previous
"""Trainium2 Bass kernel for windowed/sparse attention (nn_Attention_21732534518476).

Strategy:
  - 8 NeuronCores, one attention head per core (HEADS == 8).
  - Host-side input prep ("sharding"): transpose x -> xT, slice per-head
    projection weights, gather+transpose the relative-position bias table into
    a per-head dense [j, i] bf16 matrix, augment w_out with b_out/8.
  - On-device per core: qkv projection producing Q (q replicated at partition
    bases 0 and 32, vT at base 64) and K (k replicated at bases 0 and 32);
    transposed-score attention (scores kept [j, i] so the softmax reduction is
    a matmul with an appended ones-column on v); bias injected into PSUM via
    identity matmul; exp on the Scalar engine; unnormalized attn@v; late
    normalization; per-head output projection producing a partial outT [c, i].
    Score matmuls for the two i-halves run row-tiled (array rows 0-31 / 32-63)
    and the two attn@v matmuls run col-tiled (cols 0-32 / 64-96) so the K=32
    and M=33 matmuls overlap in the PE array.
  - Host sums the 8 partial outputs (the head reduction) and reshapes.

All matmul operands are bf16 (fp32 PSUM accumulation); exp input is fp32.
"""

import os
import sys

sys.path.insert(0, "/opt/trn_rl_repo")
os.environ.setdefault("MYCRO_LOCAL_CACHE", "1")

import numpy as np
import ml_dtypes

BF = ml_dtypes.bfloat16

B, N, C = 4, 2048, 256
HEADS, D = 8, 32
BN = B * N  # 8192
JT = 16  # j chunks of 128 per batch
IB = 16  # i blocks of 512 over the full 8192
SCALE = D ** -0.5

_CACHE = {}


def _build():
    from concourse import bass, mybir, bacc
    import concourse.tile as tile
    from concourse.masks import make_identity

    f32 = mybir.dt.float32
    bfl = mybir.dt.bfloat16
    Exp = mybir.ActivationFunctionType.Exp
    mult = mybir.AluOpType.mult

    nc = bacc.Bacc(
        "TRN2",
        target_bir_lowering=False,
        debug=False,
        num_devices=8,
    )

    xt_ext = nc.dram_tensor("xt", [128, 2, BN], bfl, kind="ExternalInput")
    # projection weights, lhsT layout [c(128), cc, m]: qv cols [q,q,vT], k cols [k,k]
    wqv_ext = nc.dram_tensor("wqv", [128, 2, 96], bfl, kind="ExternalInput")
    wk_ext = nc.dram_tensor("wk", [128, 2, 64], bfl, kind="ExternalInput")
    biast_ext = nc.dram_tensor("biast", [128, JT, N], bfl, kind="ExternalInput")
    wout_ext = nc.dram_tensor("wout", [33, 256], bfl, kind="ExternalInput")
    out_ext = nc.dram_tensor("out", [128, 2, BN], f32, kind="ExternalOutput")

    with tile.TileContext(nc) as tc:
        with (
            tc.tile_pool(name="const", bufs=1) as constp,
            tc.tile_pool(name="big", bufs=1) as bigp,
            tc.tile_pool(name="ptp", bufs=3) as ptp,
            tc.tile_pool(name="outp", bufs=2) as outp,
            tc.tile_pool(name="small", bufs=3) as smallp,
            tc.tile_pool(name="pst", bufs=3, space="PSUM") as pst,
            tc.tile_pool(name="po", bufs=2, space="PSUM") as po,
        ):
            ident = constp.tile([128, 128], bfl, tag="ident")
            make_identity(nc, ident[:])
            wqv_sb = constp.tile([128, 2, 96], bfl, tag="wqv")
            nc.sync.dma_start(out=wqv_sb[:], in_=wqv_ext[:])
            wk_sb = constp.tile([128, 2, 64], bfl, tag="wk")
            nc.sync.dma_start(out=wk_sb[:], in_=wk_ext[:])
            wout_sb = constp.tile([33, 256], bfl, tag="wout")
            nc.sync.dma_start(out=wout_sb[:], in_=wout_ext[:])

            xt_sb = bigp.tile([128, 2, BN], bfl, tag="xt")
            for cc in range(2):
                for q4 in range(4):
                    nc.sync.dma_start(
                        out=xt_sb[:, cc, q4 * 2048 : (q4 + 1) * 2048],
                        in_=xt_ext[:, cc, q4 * 2048 : (q4 + 1) * 2048],
                    )
            biast_sb = bigp.tile([128, JT, N], bfl, tag="biast")
            for jc in range(JT):
                nc.sync.dma_start(out=biast_sb[:, jc, :], in_=biast_ext[:, jc, :])

            # projections
            q_sb = bigp.tile([96, IB, 512], bfl, tag="q")  # rows: q@0, q@32, vT@64
            k_sb = bigp.tile([64, IB, 512], bfl, tag="k")  # rows: k@0, k@32
            for ib in range(IB):
                psq = pst.tile([96, 512], f32, tag="st")
                psk = pst.tile([64, 512], f32, tag="st")
                for cc in range(2):
                    nc.tensor.matmul(
                        psq[:],
                        lhsT=wqv_sb[:, cc, :],
                        rhs=xt_sb[:, cc, ib * 512 : (ib + 1) * 512],
                        start=(cc == 0),
                        stop=(cc == 1),
                    )
                for cc in range(2):
                    nc.tensor.matmul(
                        psk[:],
                        lhsT=wk_sb[:, cc, :],
                        rhs=xt_sb[:, cc, ib * 512 : (ib + 1) * 512],
                        start=(cc == 0),
                        stop=(cc == 1),
                    )
                nc.vector.tensor_copy(q_sb[:, ib, :], psq[:])
                nc.vector.tensor_copy(k_sb[:, ib, :], psk[:])

            # v in [j, d] layout with an appended ones column -> [128, b, jhi, 33]
            v1_sb = bigp.tile([128, B, JT, 33], bfl, tag="v1")
            nc.gpsimd.memset(v1_sb[:, :, :, 32:33], 1.0)
            for b in range(B):
                tp = po.tile([128, JT, 32], bfl, tag="o")
                for jh in range(JT):
                    j0 = b * N + jh * 128  # global j
                    ib = j0 // 512
                    off = j0 % 512
                    nc.tensor.transpose(
                        tp[:, jh, :],
                        q_sb[64:96, ib, off : off + 128],
                        ident[64:96, 64:96],
                    )
                nc.vector.tensor_copy(v1_sb[:, b, :, 0:32], tp[:])

            # attention units: (b, ic2) with i-halves A/B of 512 each
            for b in range(B):
                out_t = outp.tile([128, 2, 2048], f32, tag="out_t")
                for ic2 in range(2):
                    iA = b * 4 + ic2 * 2  # i-block index (512-wide) of half A
                    iB = iA + 1
                    o_pair = po.tile([128, 512], f32, tag="o")
                    o_A = o_pair
                    o_B = o_pair
                    for jc in range(JT):
                        jb = (b * N + jc * 128) // 512
                        joff = (jc * 128) % 512
                        st = pst.tile([128, 1024], f32, tag="st")
                        # bias inject (identity matmul) then scores accumulate
                        nc.tensor.matmul(
                            st[:, 0:512],
                            lhsT=ident[:],
                            rhs=biast_sb[:, jc, (iA % 4) * 512 : (iA % 4) * 512 + 512],
                            start=True,
                            stop=False,
                        )
                        nc.tensor.matmul(
                            st[:, 512:1024],
                            lhsT=ident[:],
                            rhs=biast_sb[:, jc, (iB % 4) * 512 : (iB % 4) * 512 + 512],
                            start=True,
                            stop=False,
                        )
                        # row-tiled scores: pair (0,0) and (32,0)
                        nc.tensor.matmul(
                            st[:, 0:512],
                            lhsT=k_sb[0:32, jb, joff : joff + 128],
                            rhs=q_sb[0:32, iA, :],
                            start=False,
                            stop=True,
                        )
                        nc.tensor.matmul(
                            st[:, 512:1024],
                            lhsT=k_sb[32:64, jb, joff : joff + 128],
                            rhs=q_sb[32:64, iB, :],
                            start=False,
                            stop=True,
                        )
                        pt = ptp.tile([128, 1024], bfl, tag="pt")
                        nc.scalar.activation(pt[:], st[:], Exp)
                        # col-tiled attn@v: (0,0) and (0,64)
                        nc.tensor.matmul(
                            o_A[0:33, :],
                            lhsT=v1_sb[:, b, jc, :],
                            rhs=pt[:, 0:512],
                            start=(jc == 0),
                            stop=(jc == JT - 1),
                        )
                        nc.tensor.matmul(
                            o_B[64:97, :],
                            lhsT=v1_sb[:, b, jc, :],
                            rhs=pt[:, 512:1024],
                            start=(jc == 0),
                            stop=(jc == JT - 1),
                            skip_group_check=True,
                        )
                    # tail: copy unnormalized O^T to SBUF (row 32 = sums), out
                    # projection on it, then normalize the projected tile by
                    # recip[i] on the way out (b_out/8 rides as row32*sums*recip).
                    o_sb = smallp.tile([33, 1024], bfl, tag="o_sb")
                    nc.vector.tensor_copy(o_sb[:, 0:512], o_A[0:33, :])
                    nc.vector.tensor_copy(o_sb[:, 512:1024], o_B[64:97, :])
                    recip = smallp.tile([1, 1024], f32, tag="recip")
                    nc.vector.reciprocal(recip[:, 0:512], o_A[32:33, :])
                    nc.vector.reciprocal(recip[:, 512:1024], o_B[96:97, :])
                    recipB = smallp.tile([128, 1024], f32, tag="recipB")
                    nc.gpsimd.partition_broadcast(recipB[:], recip[:])
                    for cc in range(2):
                        for half in range(2):
                            op_ps = po.tile([128, 512], f32, tag="o")
                            nc.tensor.matmul(
                                op_ps[:],
                                lhsT=wout_sb[:, cc * 128 : (cc + 1) * 128],
                                rhs=o_sb[:, half * 512 : (half + 1) * 512],
                                start=True,
                                stop=True,
                            )
                            nc.vector.tensor_tensor(
                                out_t[
                                    :,
                                    cc,
                                    ic2 * 1024 + half * 512 : ic2 * 1024 + half * 512 + 512,
                                ],
                                op_ps[:],
                                recipB[:, half * 512 : (half + 1) * 512],
                                mult,
                            )
                for cc in range(2):
                    nc.sync.dma_start(
                        out=out_ext[:, cc, b * N : (b + 1) * N],
                        in_=out_t[:, cc, :],
                    )
    nc.compile()
    return nc


def _prep_inputs(x, w_qkv, bias_table, w_out, b_out, rel_index):
    x = np.asarray(x, dtype=np.float32)
    w_qkv = np.asarray(w_qkv, dtype=np.float32)
    bias_table = np.asarray(bias_table, dtype=np.float32)
    w_out = np.asarray(w_out, dtype=np.float32)
    b_out = np.asarray(b_out, dtype=np.float32)
    rel_index = np.asarray(rel_index)

    xt = np.ascontiguousarray(
        x.reshape(BN, C).T.reshape(2, 128, BN).transpose(1, 0, 2)
    ).astype(BF)

    # rel transposed so the gather lands directly in [j, i] order
    relT = np.ascontiguousarray(rel_index.reshape(N, N).T).reshape(-1)

    in_maps = []
    for h in range(HEADS):
        wq = w_qkv[:, h * D : (h + 1) * D] * SCALE
        wk = w_qkv[:, C + h * D : C + (h + 1) * D]
        wv = w_qkv[:, 2 * C + h * D : 2 * C + (h + 1) * D]
        qv = np.concatenate([wq, wq, wv], axis=1)  # (256, 96)
        kk = np.concatenate([wk, wk], axis=1)  # (256, 64)
        wqv_h = np.ascontiguousarray(
            qv.reshape(2, 128, 96).transpose(1, 0, 2)
        ).astype(BF)
        wk_h = np.ascontiguousarray(
            kk.reshape(2, 128, 64).transpose(1, 0, 2)
        ).astype(BF)

        biast = bias_table[:, h][relT].reshape(N, N)  # [j, i]
        biast_h = np.ascontiguousarray(
            biast.reshape(JT, 128, N).transpose(1, 0, 2)
        ).astype(BF)

        wout_h = np.concatenate(
            [w_out[h * D : (h + 1) * D, :], (b_out / HEADS)[None, :]], axis=0
        ).astype(BF)  # (33, 256)

        in_maps.append(
            {
                "xt": xt,
                "wqv": wqv_h,
                "wk": wk_h,
                "biast": biast_h,
                "wout": np.ascontiguousarray(wout_h),
            }
        )
    return in_maps


def _run(in_maps, trace=False, **kwargs):
    from concourse.bass_utils import run_bass_kernel_spmd

    if "nc" not in _CACHE:
        _CACHE["nc"] = _build()
    nc = _CACHE["nc"]
    res = run_bass_kernel_spmd(
        nc, in_maps, core_ids=list(range(8)), trace=trace, **kwargs
    )
    return res


def kernel(x, w_qkv, bias_table, w_out, b_out, rel_index):
    in_maps = _prep_inputs(x, w_qkv, bias_table, w_out, b_out, rel_index)
    res = _run(in_maps, trace=False)
    acc = np.zeros((256, BN), dtype=np.float32)
    for c in range(8):
        o = res.results[c]["out"]  # (128, 2, 8192) f32
        acc += np.asarray(o).transpose(1, 0, 2).reshape(256, BN)
    out = acc.T.reshape(B, N, C).astype(np.float32)
    return out


# revision 12
# speedup vs baseline: 1.2214x; 1.0371x over previous
"""Trainium2 Bass kernel for windowed/sparse attention (nn_Attention_21732534518476).

Strategy:
  - 8 NeuronCores, one attention head per core (HEADS == 8).
  - Host-side input prep ("sharding"): transpose x -> xT, slice per-head
    projection weights, gather+transpose the relative-position bias table into
    a per-head dense [j, i] bf16 matrix, augment w_out with b_out/8.
  - On-device per core: qkv projection producing Q (q replicated at partition
    bases 0 and 32, vT at base 64) and K (k replicated at bases 0 and 32);
    transposed-score attention (scores kept [j, i] so the softmax reduction is
    a matmul with an appended ones-column on v); bias injected into PSUM via
    identity matmul; exp on the Scalar engine; unnormalized attn@v; late
    normalization; per-head output projection producing a partial outT [c, i].
    Score matmuls for the two i-halves run row-tiled (array rows 0-31 / 32-63)
    and the two attn@v matmuls run col-tiled (cols 0-32 / 64-96) so the K=32
    and M=33 matmuls overlap in the PE array.
  - Host sums the 8 partial outputs (the head reduction) and reshapes.

All matmul operands are bf16 (fp32 PSUM accumulation); exp input is fp32.
"""

import os
import sys

sys.path.insert(0, "/opt/trn_rl_repo")
os.environ.setdefault("MYCRO_LOCAL_CACHE", "1")

import numpy as np
import ml_dtypes

BF = ml_dtypes.bfloat16

B, N, C = 4, 2048, 256
HEADS, D = 8, 32
BN = B * N  # 8192
JT = 16  # j chunks of 128 per batch
IB = 16  # i blocks of 512 over the full 8192
SCALE = D ** -0.5

_CACHE = {}


def _build():
    from concourse import bass, mybir, bacc
    import concourse.tile as tile
    from concourse.masks import make_identity

    f32 = mybir.dt.float32
    bfl = mybir.dt.bfloat16
    Exp = mybir.ActivationFunctionType.Exp
    mult = mybir.AluOpType.mult

    nc = bacc.Bacc(
        "TRN2",
        target_bir_lowering=False,
        debug=False,
        num_devices=8,
    )

    xt_ext = nc.dram_tensor("xt", [128, 2, BN], bfl, kind="ExternalInput")
    # projection weights, lhsT layout [c(128), cc, m]: qv cols [q,q,vT], k cols [k,k]
    wqv_ext = nc.dram_tensor("wqv", [128, 2, 96], bfl, kind="ExternalInput")
    wk_ext = nc.dram_tensor("wk", [128, 2, 64], bfl, kind="ExternalInput")
    ebias_ext = nc.dram_tensor("ebias", [128, JT, N], bfl, kind="ExternalInput")
    wout_ext = nc.dram_tensor("wout", [33, 256], bfl, kind="ExternalInput")
    out_ext = nc.dram_tensor("out", [128, 2, BN], f32, kind="ExternalOutput")

    with tile.TileContext(nc) as tc:
        with (
            tc.tile_pool(name="const", bufs=1) as constp,
            tc.tile_pool(name="big", bufs=1) as bigp,
            tc.tile_pool(name="ptp", bufs=3) as ptp,
            tc.tile_pool(name="outp", bufs=2) as outp,
            tc.tile_pool(name="small", bufs=2) as smallp,
            tc.tile_pool(name="pst", bufs=3, space="PSUM") as pst,
            tc.tile_pool(name="po", bufs=2, space="PSUM") as po,
        ):
            ident = constp.tile([128, 128], bfl, tag="ident")
            make_identity(nc, ident[:])
            wqv_sb = constp.tile([128, 2, 96], bfl, tag="wqv")
            nc.sync.dma_start(out=wqv_sb[:], in_=wqv_ext[:])
            wk_sb = constp.tile([128, 2, 64], bfl, tag="wk")
            nc.sync.dma_start(out=wk_sb[:], in_=wk_ext[:])
            wout_sb = constp.tile([33, 256], bfl, tag="wout")
            nc.sync.dma_start(out=wout_sb[:], in_=wout_ext[:])

            xt_sb = bigp.tile([128, 2, BN], bfl, tag="xt")
            for cc in range(2):
                for q4 in range(4):
                    nc.sync.dma_start(
                        out=xt_sb[:, cc, q4 * 2048 : (q4 + 1) * 2048],
                        in_=xt_ext[:, cc, q4 * 2048 : (q4 + 1) * 2048],
                    )
            ebias_sb = bigp.tile([128, JT, N], bfl, tag="ebias")
            for jc in range(JT):
                nc.sync.dma_start(out=ebias_sb[:, jc, :], in_=ebias_ext[:, jc, :])

            # projections
            q_sb = bigp.tile([96, IB, 512], bfl, tag="q")  # rows: q@0, q@32, vT@64
            k_sb = bigp.tile([64, IB, 512], bfl, tag="k")  # rows: k@0, k@32
            for ib in range(IB):
                psq = pst.tile([96, 512], f32, tag="st")
                psk = pst.tile([64, 512], f32, tag="st")
                for cc in range(2):
                    nc.tensor.matmul(
                        psq[:],
                        lhsT=wqv_sb[:, cc, :],
                        rhs=xt_sb[:, cc, ib * 512 : (ib + 1) * 512],
                        start=(cc == 0),
                        stop=(cc == 1),
                    )
                for cc in range(2):
                    nc.tensor.matmul(
                        psk[:],
                        lhsT=wk_sb[:, cc, :],
                        rhs=xt_sb[:, cc, ib * 512 : (ib + 1) * 512],
                        start=(cc == 0),
                        stop=(cc == 1),
                    )
                nc.vector.tensor_copy(q_sb[:, ib, :], psq[:])
                nc.vector.tensor_copy(k_sb[:, ib, :], psk[:])

            # v in [j, d] layout with an appended ones column -> [128, b, jhi, 33]
            v1_sb = bigp.tile([128, B, JT, 33], bfl, tag="v1")
            nc.gpsimd.memset(v1_sb[:, :, :, 32:33], 1.0)
            for b in range(B):
                tp = po.tile([128, JT, 32], bfl, tag="o")
                for jh in range(JT):
                    j0 = b * N + jh * 128  # global j
                    ib = j0 // 512
                    off = j0 % 512
                    nc.tensor.transpose(
                        tp[:, jh, :],
                        q_sb[64:96, ib, off : off + 128],
                        ident[64:96, 64:96],
                    )
                nc.vector.tensor_copy(v1_sb[:, b, :, 0:32], tp[:])

            # attention units: (b, ic2) with i-halves A/B of 512 each
            for b in range(B):
                out_t = outp.tile([128, 2, 2048], f32, tag="out_t")
                for ic2 in range(2):
                    iA = b * 4 + ic2 * 2  # i-block index (512-wide) of half A
                    iB = iA + 1
                    o_pair = po.tile([128, 512], f32, tag="o")
                    o_A = o_pair
                    o_B = o_pair
                    for jc in range(JT):
                        jb = (b * N + jc * 128) // 512
                        joff = (jc * 128) % 512
                        st = pst.tile([128, 1024], f32, tag="st")
                        # row-tiled scores: pair (0,0) and (32,0)
                        nc.tensor.matmul(
                            st[:, 0:512],
                            lhsT=k_sb[0:32, jb, joff : joff + 128],
                            rhs=q_sb[0:32, iA, :],
                            start=True,
                            stop=True,
                        )
                        nc.tensor.matmul(
                            st[:, 512:1024],
                            lhsT=k_sb[32:64, jb, joff : joff + 128],
                            rhs=q_sb[32:64, iB, :],
                            start=True,
                            stop=True,
                        )
                        pr = ptp.tile([128, 1024], bfl, tag="pr")
                        nc.scalar.activation(pr[:], st[:], Exp)
                        pt = ptp.tile([128, 1024], bfl, tag="pt")
                        eb0 = (iA % 4) * 512
                        tt_eng = nc.vector if (jc % 4) != 3 else nc.gpsimd
                        tt_eng.tensor_tensor(
                            pt[:],
                            pr[:],
                            ebias_sb[:, jc, eb0 : eb0 + 1024],
                            mult,
                        )
                        # col-tiled attn@v: (0,0) and (0,64)
                        nc.tensor.matmul(
                            o_A[0:33, :],
                            lhsT=v1_sb[:, b, jc, :],
                            rhs=pt[:, 0:512],
                            start=(jc == 0),
                            stop=(jc == JT - 1),
                        )
                        nc.tensor.matmul(
                            o_B[64:97, :],
                            lhsT=v1_sb[:, b, jc, :],
                            rhs=pt[:, 512:1024],
                            start=(jc == 0),
                            stop=(jc == JT - 1),
                            skip_group_check=True,
                        )
                    # tail: copy unnormalized O^T to SBUF (row 32 = sums), out
                    # projection on it, then normalize the projected tile by
                    # recip[i] on the way out (b_out/8 rides as row32*sums*recip).
                    o_sb = smallp.tile([33, 1024], bfl, tag="o_sb")
                    nc.vector.tensor_copy(o_sb[:, 0:512], o_A[0:33, :])
                    nc.vector.tensor_copy(o_sb[:, 512:1024], o_B[64:97, :])
                    sums = smallp.tile([1, 1024], f32, tag="sums")
                    nc.vector.tensor_copy(sums[:, 0:512], o_A[32:33, :])
                    nc.vector.tensor_copy(sums[:, 512:1024], o_B[96:97, :])
                    recip = smallp.tile([1, 1024], f32, tag="recip")
                    nc.vector.reciprocal_approx_fast(recip[:], sums[:])
                    recipB = smallp.tile([128, 1024], f32, tag="recipB")
                    nc.gpsimd.partition_broadcast(recipB[:], recip[:])
                    for cc in range(2):
                        for half in range(2):
                            op_ps = po.tile([128, 512], f32, tag="o")
                            nc.tensor.matmul(
                                op_ps[:],
                                lhsT=wout_sb[:, cc * 128 : (cc + 1) * 128],
                                rhs=o_sb[:, half * 512 : (half + 1) * 512],
                                start=True,
                                stop=True,
                            )
                            nc.vector.tensor_tensor(
                                out_t[
                                    :,
                                    cc,
                                    ic2 * 1024 + half * 512 : ic2 * 1024 + half * 512 + 512,
                                ],
                                op_ps[:],
                                recipB[:, half * 512 : (half + 1) * 512],
                                mult,
                            )
                for cc in range(2):
                    nc.sync.dma_start(
                        out=out_ext[:, cc, b * N : (b + 1) * N],
                        in_=out_t[:, cc, :],
                    )
    nc.compile()
    return nc


def _prep_inputs(x, w_qkv, bias_table, w_out, b_out, rel_index):
    x = np.asarray(x, dtype=np.float32)
    w_qkv = np.asarray(w_qkv, dtype=np.float32)
    bias_table = np.asarray(bias_table, dtype=np.float32)
    w_out = np.asarray(w_out, dtype=np.float32)
    b_out = np.asarray(b_out, dtype=np.float32)
    rel_index = np.asarray(rel_index)

    xt = np.ascontiguousarray(
        x.reshape(BN, C).T.reshape(2, 128, BN).transpose(1, 0, 2)
    ).astype(BF)

    # rel transposed so the gather lands directly in [j, i] order
    relT = np.ascontiguousarray(rel_index.reshape(N, N).T).reshape(-1)

    in_maps = []
    for h in range(HEADS):
        wq = w_qkv[:, h * D : (h + 1) * D] * SCALE
        wk = w_qkv[:, C + h * D : C + (h + 1) * D]
        wv = w_qkv[:, 2 * C + h * D : 2 * C + (h + 1) * D]
        qv = np.concatenate([wq, wq, wv], axis=1)  # (256, 96)
        kk = np.concatenate([wk, wk], axis=1)  # (256, 64)
        wqv_h = np.ascontiguousarray(
            qv.reshape(2, 128, 96).transpose(1, 0, 2)
        ).astype(BF)
        wk_h = np.ascontiguousarray(
            kk.reshape(2, 128, 64).transpose(1, 0, 2)
        ).astype(BF)

        ebias = np.exp(bias_table[:, h][relT].reshape(N, N))  # exp(bias) [j, i]
        ebias_h = np.ascontiguousarray(
            ebias.reshape(JT, 128, N).transpose(1, 0, 2)
        ).astype(BF)

        wout_h = np.concatenate(
            [w_out[h * D : (h + 1) * D, :], (b_out / HEADS)[None, :]], axis=0
        ).astype(BF)  # (33, 256)

        in_maps.append(
            {
                "xt": xt,
                "wqv": wqv_h,
                "wk": wk_h,
                "ebias": ebias_h,
                "wout": np.ascontiguousarray(wout_h),
            }
        )
    return in_maps


def _run(in_maps, trace=False, **kwargs):
    from concourse.bass_utils import run_bass_kernel_spmd

    if "nc" not in _CACHE:
        _CACHE["nc"] = _build()
    nc = _CACHE["nc"]
    res = run_bass_kernel_spmd(
        nc, in_maps, core_ids=list(range(8)), trace=trace, **kwargs
    )
    return res


def kernel(x, w_qkv, bias_table, w_out, b_out, rel_index):
    in_maps = _prep_inputs(x, w_qkv, bias_table, w_out, b_out, rel_index)
    res = _run(in_maps, trace=False)
    acc = np.zeros((256, BN), dtype=np.float32)
    for c in range(8):
        o = res.results[c]["out"]  # (128, 2, 8192) f32
        acc += np.asarray(o).transpose(1, 0, 2).reshape(256, BN)
    out = acc.T.reshape(B, N, C).astype(np.float32)
    return out
